# revision 26
# baseline (speedup 1.0000x reference)
"""Trainium2 Bass kernel: 16-head causal attention with sink logit.

Contract: kernel(**inputs) takes the FULL inputs of the reference
(x [2,2048,1024], W_Q/W_K/W_V/W_out [1024,1024], sink [16]) and returns
the FULL output [2,2048,1024], running on 8 NeuronCores.

Sharding: core c = b*4 + g handles batch b and heads [4g, 4g+4).
Each core computes yT_partial [1024, 2048] = W_out_slice^T @ attn^T;
host sums the 4 partials per batch and transposes.
"""

import sys
import numpy as np

if "/opt/trn_rl_repo" not in sys.path:
    sys.path.insert(0, "/opt/trn_rl_repo")

B, T, C = 2, 2048, 1024
H, D = 16, 64
G = 4                # heads per core
DH = G * D           # 256 head-dims per core
NCORES = 8
QC = 512             # q chunk (matmul moving free dim)
NQ = T // QC         # 4
NKT = T // 128       # 16 k-tiles
NCC = C // 128       # 8 contraction chunks over C
SCALE = 1.0 / float(np.sqrt(D))

# vp_sb per-kt slot layout (386 cols per kt):
#   head0 (even): [V(64) | one]            off 0,   width 65,  denom row 64
#   head1 (odd):  [one | zeros(63) | V(64)] off 65,  width 128, denom row 0
#   head2 (even): [V(64) | one]            off 193, width 65,  denom row 64
#   head3 (odd):  [one | zeros(63) | V(64)] off 258, width 128, denom row 0
VP_W = 386
VP_OFF = [0, 65, 193, 258]
VP_LW = [65, 128, 65, 128]


def build_program(reps=1):
    """Build the per-core Bass program. reps>1 repeats the compute body
    (same inputs -> same outputs) for differential wall-clock timing."""
    from contextlib import ExitStack

    import concourse.bass as bass
    import concourse.tile as tile
    from concourse import bacc, mybir

    f32 = mybir.dt.float32
    f32r = mybir.dt.float32r
    bf16 = mybir.dt.bfloat16
    AF = mybir.ActivationFunctionType
    Alu = mybir.AluOpType

    nc = bacc.Bacc("TRN2", target_bir_lowering=False, debug=False)

    xt_d = nc.dram_tensor("xt", [C, T], f32r, kind="ExternalInput").ap()
    wq_d = nc.dram_tensor("wq", [C, DH], f32r, kind="ExternalInput").ap()
    wk_d = nc.dram_tensor("wk", [C, DH], f32r, kind="ExternalInput").ap()
    wv_d = nc.dram_tensor("wv", [C, DH], f32r, kind="ExternalInput").ap()
    wo_d = nc.dram_tensor("wo", [DH, C], f32r, kind="ExternalInput").ap()
    sk_d = nc.dram_tensor("sk", [1, G], f32, kind="ExternalInput").ap()
    cm_d = nc.dram_tensor("cm", [128, 4 * QC], bf16, kind="ExternalInput").ap()
    vpc_d = nc.dram_tensor("vpc", [128, NKT * 65], f32r, kind="ExternalInput").ap()
    ind_d = nc.dram_tensor("ind", [128, 128], f32r, kind="ExternalInput").ap()
    onr_d = nc.dram_tensor("onr", [1, 128], f32r, kind="ExternalInput").ap()
    yt_d = nc.dram_tensor("yt", [C, T], f32, kind="ExternalOutput").ap()

    xt_v = xt_d.rearrange("(n p) m -> p n m", p=128)   # [128, 8, 2048]
    wq_v = wq_d.rearrange("(n p) m -> p n m", p=128)   # [128, 8, 256]
    wk_v = wk_d.rearrange("(n p) m -> p n m", p=128)
    wv_v = wv_d.rearrange("(n p) m -> p n m", p=128)
    wo_v = wo_d.rearrange("(n p) m -> p n m", p=128)   # [128, 2, 1024]
    yt_v = yt_d.rearrange("(n p) m -> p n m", p=128)   # [128, 8, 2048]

    with tile.TileContext(nc) as tc, ExitStack() as ctx:
        P = lambda name, bufs: ctx.enter_context(tc.tile_pool(name=name, bufs=bufs))
        const_p = P("const", 1)
        big_p = P("big", 1)
        p_p = P("p", 4)
        y_p = P("y", 2)
        oo_p = P("oo", 1)
        row_p = P("row", 1)
        ps_p = ctx.enter_context(tc.tile_pool(name="ps", bufs=2, space="PSUM"))
        o_p = ctx.enter_context(tc.tile_pool(name="o", bufs=2, space="PSUM"))

        # ---- persistent SBUF tensors ----
        xt_sb = big_p.tile([128, NCC * T], f32r, tag="xt")           # 64KB/part
        wq_sb = big_p.tile([128, NCC * DH], f32r, tag="wq")
        wk_sb = big_p.tile([128, NCC * DH], f32r, tag="wk")
        wv_sb = big_p.tile([128, NCC * DH], f32r, tag="wv")
        wo_sb = big_p.tile([128, 2 * C], f32r, tag="wo")
        qt_sb = big_p.tile([128, 2 * T], f32r, tag="qt")
        kt_sb = big_p.tile([128, 2 * T], f32r, tag="kt")
        vp_sb = big_p.tile([128, NKT * VP_W], f32r, tag="vp")
        at_sb = big_p.tile([128, 2 * T], f32r, tag="at")             # attn^T normalized
        cm_sb = const_p.tile([128, 4 * QC], bf16, tag="cm")
        ind_sb = const_p.tile([128, 128], f32r, tag="ind")
        ones_sb = const_p.tile([128, 128], f32r, tag="ones")
        skr_sb = const_p.tile([128, G], f32, tag="skr")
        esk_sb = const_p.tile([128, G], f32, tag="esk")

        # ---- phase 0: loads + constants ----
        for i in range(NCC):
            nc.sync.dma_start(xt_sb[:, i * T:(i + 1) * T], xt_v[:, i, :])
        nc.sync.dma_start(
            wq_sb[:].rearrange("p (n m) -> p n m", m=DH), wq_v[:, :, :])
        nc.sync.dma_start(
            wk_sb[:].rearrange("p (n m) -> p n m", m=DH), wk_v[:, :, :])
        nc.sync.dma_start(
            wv_sb[:].rearrange("p (n m) -> p n m", m=DH), wv_v[:, :, :])
        nc.sync.dma_start(
            wo_sb[:].rearrange("p (n m) -> p n m", m=C), wo_v[:, :, :])
        nc.sync.dma_start(cm_sb[:, :], cm_d[:, :])
        nc.sync.dma_start(skr_sb[0:1, :], sk_d[:, :])
        nc.sync.dma_start(skr_sb[64:65, :], sk_d[:, :])
        nc.scalar.activation(esk_sb[0:1, :], skr_sb[0:1, :], AF.Exp)
        nc.scalar.activation(esk_sb[64:65, :], skr_sb[64:65, :], AF.Exp)
        nc.sync.dma_start(ones_sb[0:1, :], onr_d[:, :])
        nc.sync.dma_start(ones_sb[64:65, :], onr_d[:, :])
        # vp ones columns and zero filler ([1,1,0*63] pattern per region)
        vp_view = vp_sb[:].rearrange("p (k w) -> p k w", w=VP_W)
        vpc_view = vpc_d.rearrange("p (k w) -> p k w", w=65)
        nc.sync.dma_start(vp_view[:, :, 64:129], vpc_view[:, :, :])
        nc.sync.dma_start(vp_view[:, :, 257:322], vpc_view[:, :, :])
        nc.sync.dma_start(ind_sb[:, :], ind_d[:, :])

        for _ in range(reps):
            # ---- phase 1: Q^T and K^T projections  [d(128/pair), t] ----
            for w_sb, t_sb in ((wq_sb, qt_sb), (wk_sb, kt_sb)):
                for mt in range(2):           # head pair -> 128 d rows
                    for qp in range(NQ // 2):
                        ps = ps_p.tile([128, 2 * QC], f32, tag="ps")
                        for half in range(2):
                            qc = qp * 2 + half
                            for ci in range(NCC):
                                nc.tensor.matmul(
                                    ps[:, half * QC:(half + 1) * QC],
                                    w_sb[:, ci * DH + mt * 128: ci * DH + (mt + 1) * 128],
                                    xt_sb[:, ci * T + qc * QC: ci * T + qc * QC + QC],
                                    start=(ci == 0), stop=(ci == NCC - 1))
                        nc.vector.tensor_copy(
                            t_sb[:, mt * T + qp * 2 * QC: mt * T + (qp + 1) * 2 * QC],
                            ps[:, :])

            # ---- phase 1b: V natural [t, d] into padded vp layout ----
            for tq in range(NKT // 4):
                ps = ps_p.tile([128, 2 * QC], f32, tag="ps")
                for sub in range(4):
                    tt = tq * 4 + sub
                    for ci in range(NCC):
                        nc.tensor.matmul(
                            ps[:, sub * DH:(sub + 1) * DH],
                            xt_sb[:, ci * T + tt * 128: ci * T + (tt + 1) * 128],
                            wv_sb[:, ci * DH: (ci + 1) * DH],
                            start=(ci == 0), stop=(ci == NCC - 1))
                for sub in range(4):
                    tt = tq * 4 + sub
                    base = tt * VP_W
                    s0 = sub * DH
                    nc.vector.tensor_copy(vp_sb[:, base + 0: base + 64], ps[:, s0:s0 + 64])
                    nc.vector.tensor_copy(vp_sb[:, base + 129: base + 257], ps[:, s0 + 64:s0 + 192])
                    nc.vector.tensor_copy(vp_sb[:, base + 322: base + 386], ps[:, s0 + 192:s0 + 256])

            # ---- phase 2+3: attention per q-chunk + output projection.
            # PE runs its stream in order, so emission is software-pipelined:
            # scores(kt) are emitted before PV(kt-1), and the normalize /
            # output-projection blocks are deferred into the next kt loop.
            deferred = []

            def emit_scores(p, qc, kt):
                sAB = ps_p.tile([128, 2 * QC], f32, tag="ps")
                nc.tensor.matmul(
                    sAB[:, 0:QC],
                    kt_sb[0:64, p * T + kt * 128: p * T + (kt + 1) * 128],
                    qt_sb[0:64, p * T + qc * QC: p * T + qc * QC + QC],
                    start=True, stop=True)
                nc.tensor.matmul(
                    sAB[:, QC:2 * QC],
                    kt_sb[64:128, p * T + kt * 128: p * T + (kt + 1) * 128],
                    qt_sb[64:128, p * T + qc * QC: p * T + qc * QC + QC],
                    start=True, stop=True)
                diag = kt - 4 * qc
                pAB = p_p.tile([128, 2 * QC], f32r, tag="p")
                nc.scalar.activation(pAB[:, :], sAB[:, :], AF.Exp, scale=SCALE)
                if diag >= 0:
                    msk = cm_sb[:, diag * QC:(diag + 1) * QC]
                    with nc.allow_low_precision(reason="0/1 mask mult"):
                        nc.vector.tensor_mul(pAB[:, 0:QC], pAB[:, 0:QC], msk)
                        nc.gpsimd.tensor_mul(pAB[:, QC:2 * QC], pAB[:, QC:2 * QC], msk)
                return pAB

            def emit_pv(p, qc, kt, nkt, oAB, pAB):
                hA, hB = 2 * p, 2 * p + 1
                base = kt * VP_W
                nc.tensor.matmul(
                    oAB[0:65, 0:QC],
                    vp_sb[:, base + VP_OFF[hA]: base + VP_OFF[hA] + 65],
                    pAB[:, 0:QC],
                    start=(kt == 0), stop=(kt == nkt - 1))
                nc.tensor.matmul(
                    oAB[:, QC:2 * QC],
                    vp_sb[:, base + VP_OFF[hB]: base + VP_OFF[hB] + 128],
                    pAB[:, QC:2 * QC],
                    start=(kt == 0), stop=(kt == nkt - 1))

            def make_normalize(p, qc, oAB):
                def emit():
                    hA, hB = 2 * p, 2 * p + 1
                    oo = oo_p.tile([128, 2 * QC], f32, tag="oo")
                    nc.vector.tensor_copy(oo[0:65, 0:QC], oAB[0:65, 0:QC])
                    nc.vector.tensor_copy(oo[:, QC:2 * QC], oAB[:, QC:2 * QC])
                    dn = row_p.tile([128, QC], f32, tag="row")
                    rc = row_p.tile([128, QC], f32r, tag="rowr")
                    bc = ps_p.tile([128, 2 * QC], f32, tag="ps")
                    nc.vector.tensor_scalar(
                        out=dn[64:65, :], in0=oo[64:65, 0:QC],
                        scalar1=esk_sb[64:65, hA:hA + 1], scalar2=None, op0=Alu.add)
                    nc.vector.tensor_scalar(
                        out=dn[0:1, :], in0=oo[0:1, QC:2 * QC],
                        scalar1=esk_sb[0:1, hB:hB + 1], scalar2=None, op0=Alu.add)
                    with nc.allow_low_precision(reason="f32r recip for PE broadcast"):
                        nc.vector.reciprocal(rc[64:65, :], dn[64:65, :])
                        nc.vector.reciprocal(rc[0:1, :], dn[0:1, :])
                    nc.tensor.matmul(
                        bc[:, 0:QC], ind_sb[64:65, :], rc[64:65, :],
                        start=True, stop=True)
                    nc.tensor.matmul(
                        bc[:, QC:2 * QC], ind_sb[0:1, :], rc[0:1, :],
                        start=True, stop=True)
                    nc.vector.tensor_mul(
                        at_sb[0:64, p * T + qc * QC: p * T + qc * QC + QC],
                        oo[0:64, 0:QC], bc[0:64, 0:QC])
                    nc.vector.tensor_mul(
                        at_sb[64:128, p * T + qc * QC: p * T + qc * QC + QC],
                        oo[64:128, QC:2 * QC], bc[64:128, QC:2 * QC])
                return emit

            def make_wout(qc, cop):
                def emit():
                    ps = ps_p.tile([128, 2 * QC], f32, tag="ps")
                    for half in range(2):
                        co = cop * 2 + half
                        for j in range(2):
                            nc.tensor.matmul(
                                ps[:, half * QC:(half + 1) * QC],
                                wo_sb[:, j * C + co * 128: j * C + (co + 1) * 128],
                                at_sb[:, j * T + qc * QC: j * T + qc * QC + QC],
                                start=(j == 0), stop=(j == 1))
                    yt = y_p.tile([128, 2 * QC], f32, tag="y")
                    nc.vector.tensor_copy(yt[:, :], ps[:, :])
                    nc.sync.dma_start(
                        yt_v[:, cop * 2: cop * 2 + 2, qc * QC: qc * QC + QC],
                        yt[:, :].rearrange("p (n m) -> p n m", m=QC))
                return emit

            for qc in range(NQ):
                nkt = 4 * qc + 4
                for p in range(2):
                    oAB = o_p.tile([128, 2 * QC], f32, tag="o")
                    prev2 = emit_scores(p, qc, 0)
                    prev1 = emit_scores(p, qc, 1)
                    for kt in range(2, nkt):
                        cur = emit_scores(p, qc, kt)
                        if deferred:
                            deferred.pop(0)()
                        emit_pv(p, qc, kt - 2, nkt, oAB, prev2)
                        prev2, prev1 = prev1, cur
                    emit_pv(p, qc, nkt - 2, nkt, oAB, prev2)
                    emit_pv(p, qc, nkt - 1, nkt, oAB, prev1)
                    deferred.append(make_normalize(p, qc, oAB))
                for cop in range(NCC // 2):
                    deferred.append(make_wout(qc, cop))
            for fn in deferred:
                fn()
            deferred.clear()

    nc.compile()
    return nc


def make_causal_masks():
    import ml_dtypes
    cm = np.zeros((128, 4 * QC), dtype=np.float32)
    kl = np.arange(128)[:, None]
    ql = np.arange(QC)[None, :]
    for m in range(4):
        cm[:, m * QC:(m + 1) * QC] = (ql >= kl + 128 * m).astype(np.float32)
    return cm.astype(ml_dtypes.bfloat16)


def shard_inputs(x, W_Q, W_K, W_V, W_out, sink):
    cm = make_causal_masks()
    vpc = np.zeros((128, 65), dtype=np.float32)
    vpc[:, 0:2] = 1.0
    vpc = np.tile(vpc, (1, NKT))
    ind = np.zeros((128, 128), dtype=np.float32)
    ind[64, 0:64] = 1.0   # head A recip (row 64) -> rows 0-63
    ind[0, 64:128] = 1.0  # head B recip (row 0) -> rows 64-127
    in_maps = []
    for c in range(NCORES):
        b, g = divmod(c, G)
        cols = slice(g * DH, (g + 1) * DH)
        in_maps.append({
            "xt": np.ascontiguousarray(x[b].T),
            "wq": np.ascontiguousarray(W_Q[:, cols]),
            "wk": np.ascontiguousarray(W_K[:, cols]),
            "wv": np.ascontiguousarray(W_V[:, cols]),
            "wo": np.ascontiguousarray(W_out[cols, :]),
            "sk": np.ascontiguousarray(sink[g * G:(g + 1) * G][None, :]),
            "cm": cm,
            "vpc": vpc,
            "ind": ind,
            "onr": np.ones((1, 128), dtype=np.float32),
        })
    return in_maps


def gather_outputs(results):
    out = np.zeros((B, T, C), dtype=np.float32)
    for b in range(B):
        acc = np.zeros((C, T), dtype=np.float32)
        for g in range(G):
            acc += results[b * G + g]["yt"]
        out[b] = acc.T
    return out


_CACHE = {}


def _get_program():
    if "nc" not in _CACHE:
        _CACHE["nc"] = build_program(reps=1)
    return _CACHE["nc"]


def kernel(x, W_Q, W_K, W_V, W_out, sink):
    from concourse.bass_utils import run_bass_kernel_spmd

    x = np.asarray(x, dtype=np.float32)
    W_Q = np.asarray(W_Q, dtype=np.float32)
    W_K = np.asarray(W_K, dtype=np.float32)
    W_V = np.asarray(W_V, dtype=np.float32)
    W_out = np.asarray(W_out, dtype=np.float32)
    sink = np.asarray(sink, dtype=np.float32)

    nc = _get_program()
    in_maps = shard_inputs(x, W_Q, W_K, W_V, W_out, sink)
    res = run_bass_kernel_spmd(nc, in_maps, core_ids=list(range(NCORES)))
    return gather_outputs(res.results)


# revision 27
# speedup vs baseline: 1.6189x; 1.6189x over previous
"""Trainium2 Bass kernel: 16-head causal attention with sink logit.

Contract: kernel(**inputs) takes the FULL inputs of the reference
(x [2,2048,1024], W_Q/W_K/W_V/W_out [1024,1024], sink [16]) and returns
the FULL output [2,2048,1024], running on 8 NeuronCores.

Sharding: core c = b*4 + g handles batch b and heads [4g, 4g+4).
Each core computes yT_partial [1024, 2048] = W_out_slice^T @ attn^T;
host sums the 4 partials per batch and transposes.
"""

import sys
import numpy as np

if "/opt/trn_rl_repo" not in sys.path:
    sys.path.insert(0, "/opt/trn_rl_repo")

B, T, C = 2, 2048, 1024
H, D = 16, 64
G = 4                # heads per core
DH = G * D           # 256 head-dims per core
NCORES = 8
QC = 512             # q chunk (matmul moving free dim)
NQ = T // QC         # 4
NKT = T // 128       # 16 k-tiles
NCC = C // 128       # 8 contraction chunks over C
SCALE = 1.0 / float(np.sqrt(D))

# vp_sb per-kt slot layout (386 cols per kt):
#   head0 (even): [V(64) | one]            off 0,   width 65,  denom row 64
#   head1 (odd):  [one | zeros(63) | V(64)] off 65,  width 128, denom row 0
#   head2 (even): [V(64) | one]            off 193, width 65,  denom row 64
#   head3 (odd):  [one | zeros(63) | V(64)] off 258, width 128, denom row 0
VP_W = 386
VP_OFF = [0, 65, 193, 258]
VP_LW = [65, 128, 65, 128]


def build_program(reps=1):
    """Build the per-core Bass program. reps>1 repeats the compute body
    (same inputs -> same outputs) for differential wall-clock timing."""
    from contextlib import ExitStack

    import concourse.bass as bass
    import concourse.tile as tile
    from concourse import bacc, mybir

    f32 = mybir.dt.float32
    f32r = mybir.dt.float32r
    bf16 = mybir.dt.bfloat16
    AF = mybir.ActivationFunctionType
    Alu = mybir.AluOpType

    nc = bacc.Bacc("TRN2", target_bir_lowering=False, debug=False)

    xt_d = nc.dram_tensor("xt", [C, T], f32r, kind="ExternalInput").ap()
    wq_d = nc.dram_tensor("wq", [C, DH], f32r, kind="ExternalInput").ap()
    wk_d = nc.dram_tensor("wk", [C, DH], f32r, kind="ExternalInput").ap()
    wv_d = nc.dram_tensor("wv", [C, DH], f32r, kind="ExternalInput").ap()
    wo_d = nc.dram_tensor("wo", [DH, C], f32r, kind="ExternalInput").ap()
    sk_d = nc.dram_tensor("sk", [1, G], f32, kind="ExternalInput").ap()
    cm_d = nc.dram_tensor("cm", [128, 4 * QC], bf16, kind="ExternalInput").ap()
    vpc_d = nc.dram_tensor("vpc", [128, NKT * 65], f32r, kind="ExternalInput").ap()
    ind_d = nc.dram_tensor("ind", [128, 128], f32r, kind="ExternalInput").ap()
    onr_d = nc.dram_tensor("onr", [1, 128], f32r, kind="ExternalInput").ap()
    yt_d = nc.dram_tensor("yt", [C, T], f32, kind="ExternalOutput").ap()

    xt_v = xt_d.rearrange("(n p) m -> p n m", p=128)   # [128, 8, 2048]
    wq_v = wq_d.rearrange("(n p) m -> p n m", p=128)   # [128, 8, 256]
    wk_v = wk_d.rearrange("(n p) m -> p n m", p=128)
    wv_v = wv_d.rearrange("(n p) m -> p n m", p=128)
    wo_v = wo_d.rearrange("(n p) m -> p n m", p=128)   # [128, 2, 1024]
    yt_v = yt_d.rearrange("(n p) m -> p n m", p=128)   # [128, 8, 2048]

    with tile.TileContext(nc) as tc, ExitStack() as ctx:
        P = lambda name, bufs: ctx.enter_context(tc.tile_pool(name=name, bufs=bufs))
        const_p = P("const", 1)
        big_p = P("big", 1)
        p_p = P("p", 4)
        y_p = P("y", 2)
        oo_p = P("oo", 1)
        row_p = P("row", 1)
        ps_p = ctx.enter_context(tc.tile_pool(name="ps", bufs=2, space="PSUM"))
        o_p = ctx.enter_context(tc.tile_pool(name="o", bufs=2, space="PSUM"))

        # ---- persistent SBUF tensors ----
        xt_sb = big_p.tile([128, NCC * T], f32r, tag="xt")           # 64KB/part
        wq_sb = big_p.tile([128, NCC * DH], f32r, tag="wq")
        wk_sb = big_p.tile([128, NCC * DH], f32r, tag="wk")
        wv_sb = big_p.tile([128, NCC * DH], f32r, tag="wv")
        wo_sb = big_p.tile([128, 2 * C], f32r, tag="wo")
        qt_sb = big_p.tile([128, 2 * T], f32r, tag="qt")
        kt_sb = big_p.tile([128, 2 * T], f32r, tag="kt")
        vp_sb = big_p.tile([128, NKT * VP_W], f32r, tag="vp")
        at_sb = big_p.tile([128, 2 * T], f32r, tag="at")             # attn^T normalized
        cm_sb = const_p.tile([128, 4 * QC], bf16, tag="cm")
        ind_sb = const_p.tile([128, 128], f32r, tag="ind")
        ones_sb = const_p.tile([128, 128], f32r, tag="ones")
        skr_sb = const_p.tile([128, G], f32, tag="skr")
        esk_sb = const_p.tile([128, G], f32, tag="esk")

        # ---- phase 0: loads + constants ----
        for i in range(NCC):
            nc.sync.dma_start(xt_sb[:, i * T:(i + 1) * T], xt_v[:, i, :])
        nc.sync.dma_start(
            wq_sb[:].rearrange("p (n m) -> p n m", m=DH), wq_v[:, :, :])
        nc.sync.dma_start(
            wk_sb[:].rearrange("p (n m) -> p n m", m=DH), wk_v[:, :, :])
        nc.sync.dma_start(
            wv_sb[:].rearrange("p (n m) -> p n m", m=DH), wv_v[:, :, :])
        nc.sync.dma_start(
            wo_sb[:].rearrange("p (n m) -> p n m", m=C), wo_v[:, :, :])
        nc.sync.dma_start(cm_sb[:, :], cm_d[:, :])
        nc.sync.dma_start(skr_sb[0:1, :], sk_d[:, :])
        nc.sync.dma_start(skr_sb[64:65, :], sk_d[:, :])
        nc.scalar.activation(esk_sb[0:1, :], skr_sb[0:1, :], AF.Exp)
        nc.scalar.activation(esk_sb[64:65, :], skr_sb[64:65, :], AF.Exp)
        nc.sync.dma_start(ones_sb[0:1, :], onr_d[:, :])
        nc.sync.dma_start(ones_sb[64:65, :], onr_d[:, :])
        # vp ones columns and zero filler ([1,1,0*63] pattern per region)
        vp_view = vp_sb[:].rearrange("p (k w) -> p k w", w=VP_W)
        vpc_view = vpc_d.rearrange("p (k w) -> p k w", w=65)
        nc.sync.dma_start(vp_view[:, :, 64:129], vpc_view[:, :, :])
        nc.sync.dma_start(vp_view[:, :, 257:322], vpc_view[:, :, :])
        nc.sync.dma_start(ind_sb[:, :], ind_d[:, :])

        for _ in range(reps):
            # ---- phase 1: Q^T and K^T projections  [d(128/pair), t] ----
            for w_sb, t_sb in ((wq_sb, qt_sb), (wk_sb, kt_sb)):
                for mt in range(2):           # head pair -> 128 d rows
                    for qp in range(NQ // 2):
                        ps = ps_p.tile([128, 2 * QC], f32, tag="ps")
                        for half in range(2):
                            qc = qp * 2 + half
                            for ci in range(NCC):
                                nc.tensor.matmul(
                                    ps[:, half * QC:(half + 1) * QC],
                                    w_sb[:, ci * DH + mt * 128: ci * DH + (mt + 1) * 128],
                                    xt_sb[:, ci * T + qc * QC: ci * T + qc * QC + QC],
                                    start=(ci == 0), stop=(ci == NCC - 1))
                        nc.vector.tensor_copy(
                            t_sb[:, mt * T + qp * 2 * QC: mt * T + (qp + 1) * 2 * QC],
                            ps[:, :])

            # ---- phase 1b: V natural [t, d] into padded vp layout ----
            for tq in range(NKT // 4):
                ps = ps_p.tile([128, 2 * QC], f32, tag="ps")
                for sub in range(4):
                    tt = tq * 4 + sub
                    for ci in range(NCC):
                        nc.tensor.matmul(
                            ps[:, sub * DH:(sub + 1) * DH],
                            xt_sb[:, ci * T + tt * 128: ci * T + (tt + 1) * 128],
                            wv_sb[:, ci * DH: (ci + 1) * DH],
                            start=(ci == 0), stop=(ci == NCC - 1))
                for sub in range(4):
                    tt = tq * 4 + sub
                    base = tt * VP_W
                    s0 = sub * DH
                    nc.vector.tensor_copy(vp_sb[:, base + 0: base + 64], ps[:, s0:s0 + 64])
                    nc.vector.tensor_copy(vp_sb[:, base + 129: base + 257], ps[:, s0 + 64:s0 + 192])
                    nc.vector.tensor_copy(vp_sb[:, base + 322: base + 386], ps[:, s0 + 192:s0 + 256])

            # ---- phase 2+3: attention per q-chunk + output projection.
            # PE runs its stream in order, so emission is software-pipelined:
            # scores(kt) are emitted before PV(kt-1), and the normalize /
            # output-projection blocks are deferred into the next kt loop.
            deferred = []

            def emit_scores(p, qc, kt):
                sAB = ps_p.tile([128, 2 * QC], f32, tag="ps")
                nc.tensor.matmul(
                    sAB[:, 0:QC],
                    kt_sb[0:64, p * T + kt * 128: p * T + (kt + 1) * 128],
                    qt_sb[0:64, p * T + qc * QC: p * T + qc * QC + QC],
                    start=True, stop=True)
                nc.tensor.matmul(
                    sAB[:, QC:2 * QC],
                    kt_sb[64:128, p * T + kt * 128: p * T + (kt + 1) * 128],
                    qt_sb[64:128, p * T + qc * QC: p * T + qc * QC + QC],
                    start=True, stop=True)
                diag = kt - 4 * qc
                pAB = p_p.tile([128, 2 * QC], f32r, tag="p")
                nc.scalar.activation(pAB[:, :], sAB[:, :], AF.Exp, scale=SCALE)
                if diag >= 0:
                    msk = cm_sb[:, diag * QC:(diag + 1) * QC]
                    with nc.allow_low_precision(reason="0/1 mask mult"):
                        nc.vector.tensor_mul(pAB[:, 0:QC], pAB[:, 0:QC], msk)
                        nc.gpsimd.tensor_mul(pAB[:, QC:2 * QC], pAB[:, QC:2 * QC], msk)
                return pAB

            def emit_pv(p, qc, kt, nkt, oAB, pAB):
                hA, hB = 2 * p, 2 * p + 1
                base = kt * VP_W
                nc.tensor.matmul(
                    oAB[0:65, 0:QC],
                    vp_sb[:, base + VP_OFF[hA]: base + VP_OFF[hA] + 65],
                    pAB[:, 0:QC],
                    start=(kt == 0), stop=(kt == nkt - 1))
                nc.tensor.matmul(
                    oAB[:, QC:2 * QC],
                    vp_sb[:, base + VP_OFF[hB]: base + VP_OFF[hB] + 128],
                    pAB[:, QC:2 * QC],
                    start=(kt == 0), stop=(kt == nkt - 1))

            def make_normalize(p, qc, oAB):
                def emit():
                    hA, hB = 2 * p, 2 * p + 1
                    oo = oo_p.tile([128, 2 * QC], f32, tag="oo")
                    nc.vector.tensor_copy(oo[0:65, 0:QC], oAB[0:65, 0:QC])
                    nc.vector.tensor_copy(oo[:, QC:2 * QC], oAB[:, QC:2 * QC])
                    dn = row_p.tile([128, QC], f32, tag="row")
                    rc = row_p.tile([128, QC], f32r, tag="rowr")
                    bc = ps_p.tile([128, 2 * QC], f32, tag="ps")
                    nc.vector.tensor_scalar(
                        out=dn[64:65, :], in0=oo[64:65, 0:QC],
                        scalar1=esk_sb[64:65, hA:hA + 1], scalar2=None, op0=Alu.add)
                    nc.vector.tensor_scalar(
                        out=dn[0:1, :], in0=oo[0:1, QC:2 * QC],
                        scalar1=esk_sb[0:1, hB:hB + 1], scalar2=None, op0=Alu.add)
                    with nc.allow_low_precision(reason="f32r recip for PE broadcast"):
                        nc.vector.reciprocal(rc[64:65, :], dn[64:65, :])
                        nc.vector.reciprocal(rc[0:1, :], dn[0:1, :])
                    nc.tensor.matmul(
                        bc[:, 0:QC], ind_sb[64:65, :], rc[64:65, :],
                        start=True, stop=True)
                    nc.tensor.matmul(
                        bc[:, QC:2 * QC], ind_sb[0:1, :], rc[0:1, :],
                        start=True, stop=True)
                    nc.vector.tensor_mul(
                        at_sb[0:64, p * T + qc * QC: p * T + qc * QC + QC],
                        oo[0:64, 0:QC], bc[0:64, 0:QC])
                    nc.vector.tensor_mul(
                        at_sb[64:128, p * T + qc * QC: p * T + qc * QC + QC],
                        oo[64:128, QC:2 * QC], bc[64:128, QC:2 * QC])
                return emit

            def make_wout(qc, cop):
                def emit():
                    ps = ps_p.tile([128, 2 * QC], f32, tag="ps")
                    for half in range(2):
                        co = cop * 2 + half
                        for j in range(2):
                            nc.tensor.matmul(
                                ps[:, half * QC:(half + 1) * QC],
                                wo_sb[:, j * C + co * 128: j * C + (co + 1) * 128],
                                at_sb[:, j * T + qc * QC: j * T + qc * QC + QC],
                                start=(j == 0), stop=(j == 1))
                    yt = y_p.tile([128, 2 * QC], f32, tag="y")
                    nc.vector.tensor_copy(yt[:, :], ps[:, :])
                    nc.sync.dma_start(
                        yt_v[:, cop * 2: cop * 2 + 2, qc * QC: qc * QC + QC],
                        yt[:, :].rearrange("p (n m) -> p n m", m=QC))
                return emit

            for qc in range(NQ):
                nkt = 4 * qc + 4
                for p in range(2):
                    oAB = o_p.tile([128, 2 * QC], f32, tag="o")
                    prev = emit_scores(p, qc, 0)
                    for kt in range(1, nkt):
                        cur = emit_scores(p, qc, kt)
                        if kt >= 2 and deferred:
                            deferred.pop(0)()
                        emit_pv(p, qc, kt - 1, nkt, oAB, prev)
                        prev = cur
                    emit_pv(p, qc, nkt - 1, nkt, oAB, prev)
                    deferred.append(make_normalize(p, qc, oAB))
                for cop in range(NCC // 2):
                    deferred.append(make_wout(qc, cop))
            for fn in deferred:
                fn()
            deferred.clear()

    nc.compile()
    return nc


def make_causal_masks():
    import ml_dtypes
    cm = np.zeros((128, 4 * QC), dtype=np.float32)
    kl = np.arange(128)[:, None]
    ql = np.arange(QC)[None, :]
    for m in range(4):
        cm[:, m * QC:(m + 1) * QC] = (ql >= kl + 128 * m).astype(np.float32)
    return cm.astype(ml_dtypes.bfloat16)


def shard_inputs(x, W_Q, W_K, W_V, W_out, sink):
    cm = make_causal_masks()
    vpc = np.zeros((128, 65), dtype=np.float32)
    vpc[:, 0:2] = 1.0
    vpc = np.tile(vpc, (1, NKT))
    ind = np.zeros((128, 128), dtype=np.float32)
    ind[64, 0:64] = 1.0   # head A recip (row 64) -> rows 0-63
    ind[0, 64:128] = 1.0  # head B recip (row 0) -> rows 64-127
    in_maps = []
    for c in range(NCORES):
        b, g = divmod(c, G)
        cols = slice(g * DH, (g + 1) * DH)
        in_maps.append({
            "xt": np.ascontiguousarray(x[b].T),
            "wq": np.ascontiguousarray(W_Q[:, cols]),
            "wk": np.ascontiguousarray(W_K[:, cols]),
            "wv": np.ascontiguousarray(W_V[:, cols]),
            "wo": np.ascontiguousarray(W_out[cols, :]),
            "sk": np.ascontiguousarray(sink[g * G:(g + 1) * G][None, :]),
            "cm": cm,
            "vpc": vpc,
            "ind": ind,
            "onr": np.ones((1, 128), dtype=np.float32),
        })
    return in_maps


def gather_outputs(results):
    out = np.zeros((B, T, C), dtype=np.float32)
    for b in range(B):
        acc = np.zeros((C, T), dtype=np.float32)
        for g in range(G):
            acc += results[b * G + g]["yt"]
        out[b] = acc.T
    return out


_CACHE = {}


def _get_program():
    if "nc" not in _CACHE:
        _CACHE["nc"] = build_program(reps=1)
    return _CACHE["nc"]


def kernel(x, W_Q, W_K, W_V, W_out, sink):
    from concourse.bass_utils import run_bass_kernel_spmd

    x = np.asarray(x, dtype=np.float32)
    W_Q = np.asarray(W_Q, dtype=np.float32)
    W_K = np.asarray(W_K, dtype=np.float32)
    W_V = np.asarray(W_V, dtype=np.float32)
    W_out = np.asarray(W_out, dtype=np.float32)
    sink = np.asarray(sink, dtype=np.float32)

    nc = _get_program()
    in_maps = shard_inputs(x, W_Q, W_K, W_V, W_out, sink)
    res = run_bass_kernel_spmd(nc, in_maps, core_ids=list(range(NCORES)))
    return gather_outputs(res.results)
